# revision 15
# baseline (speedup 1.0000x reference)
"""Trainium2 Bass kernel: 3D 'same' convolution (implicit GEMM).

Problem: x (4, 64, 24, 24, 24) f32, weight (1, 128, 1728) f32
         -> out (4, 128, 24, 24, 24) f32  (SAME conv3d, k=3)

Sharding (8 cores): batch (4) x z-halves (2). Each core computes
out[b, :, z0:z0+12] for its (b, zh) shard; no inter-core communication.

Per-core algorithm: tap-PAIR-packed implicit GEMM in bf16 (fp32 PSUM).
The PE drains one 128-wide output column per cycle regardless of row
tiling, so throughput is (matmul instructions x N columns) -- the win
is packing TWO conv taps into each 128-deep contraction: partitions
0-63 hold the zero-padded input window, partitions 64-127 hold the
SAME window translated by one voxel, so a single K=128 matmul
accumulates both taps of a pair. Buffer 0 pairs along x (9 pairs),
buffer 1 along y at dx=2 (3 pairs); the 3 leftover taps (dz,2,2) are
K=64 matmuls on the lower half. 15 column-drain units instead of 27.
The shifted replicas are prebuilt host-side so each buffer is a single
whole-tile HBM load.

Loop order is unit-outer over groups of output tiles (one PSUM bank
per tile, 8 banks), so each stationary weight is reused for up to 8
consecutive matmuls; evacuation is a single DVE copy per tile (no
cross-bank add). For timing, the body is unrolled UNROLL x inside the
hardware For_i loop and software-pipelined over NPAR parity buffer
sets: body k prefetches sub-iteration k+2's inputs while computing on
k's, giving loads a two-sub-iteration window.
"""

import sys

if "/opt/trn_rl_repo" not in sys.path:
    sys.path.insert(0, "/opt/trn_rl_repo")

import numpy as np

CIN, COUT, K = 64, 128, 3
DHW = 24  # cubic spatial extent
ZS = 12  # z-planes per shard
NP = 14  # padded z-planes per shard window (ZS + 2 halo)
PW = 26  # padded y/x extent
N_CORES = 8
UNROLL = 8
NPAR = 4  # parity buffer sets (UNROLL % NPAR == 0)

# pair units: (buffer j, lower-tap (dz, dy, dx)); buffer j's upper half
# is shifted by +1 in x (j=0) or y (j=1), pairing the lower tap with
# (dz,dy,dx+1) / (dz,dy+1,dx). Units 12-14 are the unpaired taps
# (dz,2,2), computed as K=64 matmuls on the lower half only.
UNITS = [(0, dz, dy, 0) for dz in range(3) for dy in range(3)] + [
    (1, dz, 0, 2) for dz in range(3)
]
SINGLES = [(0, 2, 2), (1, 2, 2), (2, 2, 2)]
N_UNITS = 15


def _build_program(loop_n=None):
    """Build the SPMD Bass program (one NeuronCore's view).

    loop_n: if set, wrap the whole body in a hardware For_i loop with
    loop_n/UNROLL iterations of UNROLL unrolled bodies (used by test.py
    for wall-clock timing). Must be divisible by UNROLL.
    """
    import concourse.tile as tile
    from concourse import bacc, mybir

    F32 = mybir.dt.float32
    BF16 = mybir.dt.bfloat16

    nc = bacc.Bacc("TRN2")
    x_in = nc.declare_dram_parameter("x", [2, 128, NP, PW, PW], BF16, isOutput=False)
    wk_in = nc.declare_dram_parameter("wk", [128, N_UNITS, 128], BF16, isOutput=False)
    y_out = nc.declare_dram_parameter("y", [128, ZS, DHW, DHW], BF16, isOutput=True)

    with tile.TileContext(nc) as tc:
        with (
            tc.tile_pool(name="wp", bufs=1) as w_pool,
            tc.tile_pool(name="xp", bufs=1) as x_pool,
            tc.tile_pool(name="ps", bufs=8, space="PSUM") as ps_pool,
            tc.tile_pool(name="ob", bufs=4) as ob_pool,
        ):
            # weights are loop-invariant: load once, outside the loop
            W = w_pool.tile([128, N_UNITS, 128], BF16, name="W")
            nc.sync.dma_start(out=W[:], in_=wk_in[:])

            # NPAR explicit buffer sets; the loop body prefetches parity
            # (k+2) % NPAR while computing on parity k % NPAR
            XJP = [
                [
                    x_pool.tile([128, NP, PW, PW], BF16, name=f"XJ{j}p{p}")
                    for j in range(2)
                ]
                for p in range(NPAR)
            ]

            def load_set(XJ):
                # both input loads on the sync ring: the scalar ring's
                # head blocks on evac-gated output DMAs (head-of-line),
                # which would delay a load queued behind them
                nc.sync.dma_start(out=XJ[0][:], in_=x_in[0])
                nc.sync.dma_start(out=XJ[1][:], in_=x_in[1])

            def compute(XJ):
                # output tiles: ("plane", z) N=504 (21x24, 2D AP)
                #           or ("rem", zoff) N=432 (6x3x24, 3D AP)
                groups = (
                    [("plane", z) for z in range(8)],
                    [("plane", z) for z in range(8, 12)] + [("rem", 0), ("rem", 6)],
                )

                def rhs(kind, z, j, dz, dy, dx, lo, hi):
                    if kind == "plane":
                        return XJ[j][lo:hi, z + dz, dy : dy + 21, dx : dx + 24]
                    return XJ[j][
                        lo:hi, z + dz : z + dz + 6, 21 + dy : 24 + dy, dx : dx + 24
                    ]

                for group in groups:
                    n_full = [504 if kind == "plane" else 432 for kind, _ in group]
                    ps = [
                        ps_pool.tile([128, 512], F32, name="ps", tag="ps")
                        for _ in group
                    ]
                    for u in range(N_UNITS):
                        if u < 12:
                            j, dz, dy, dx = UNITS[u]
                            for t, (kind, z) in enumerate(group):
                                nc.tensor.matmul(
                                    ps[t][:, : n_full[t]],
                                    lhsT=W[0:128, u, :],
                                    rhs=rhs(kind, z, j, dz, dy, dx, 0, 128),
                                    start=(u == 0),
                                    stop=False,
                                    skip_group_check=True,
                                )
                        else:
                            # unpaired tap: K=64, lower half only
                            dz, dy, dx = SINGLES[u - 12]
                            for t, (kind, z) in enumerate(group):
                                nc.tensor.matmul(
                                    ps[t][:, : n_full[t]],
                                    lhsT=W[0:64, u, :],
                                    rhs=rhs(kind, z, 0, dz, dy, dx, 0, 64),
                                    start=False,
                                    stop=(u == N_UNITS - 1),
                                    skip_group_check=True,
                                    tile_position=(0, 0),
                                )

                    for t, (kind, z) in enumerate(group):
                        n = n_full[t]
                        # bf16 evacuation: halves out-DMA bytes and doubles
                        # DVE copy throughput; host upcasts to f32
                        ob = ob_pool.tile([128, 512], BF16, name="ob", tag="ob")
                        nc.vector.tensor_copy(ob[:, :n], ps[t][:, :n])
                        if kind == "plane":
                            nc.scalar.dma_start(
                                out=y_out[:, z, 0:21, :], in_=ob[:, :n]
                            )
                        else:
                            nc.scalar.dma_start(
                                out=y_out[:, z : z + 6, 21:24, :], in_=ob[:, :n]
                            )

            if loop_n is not None:
                # pick the largest unroll dividing loop_n; prefetch
                # distance shrinks gracefully for small unrolls
                unroll = next(u for u in (8, 4, 2, 1) if loop_n % u == 0)
                npar = min(NPAR, unroll)
                pf = 2 if npar >= 4 else (1 if npar == 2 else 0)
                for p in range(max(pf, 1)):
                    load_set(XJP[p])
                with tc.For_i(0, loop_n // unroll, 1, staggered_reset=True) as _i:
                    for k in range(unroll):
                        if pf:
                            load_set(XJP[(k + pf) % npar])
                        compute(XJP[k % npar])
                        if not pf:
                            load_set(XJP[0])
            else:
                load_set(XJP[0])
                compute(XJP[0])

    nc.finalize()
    return nc


def _make_in_maps(x, weight):
    import ml_dtypes

    bf16 = ml_dtypes.bfloat16
    w = np.asarray(weight, np.float32).reshape(COUT, CIN, K, K, K)
    wk = np.zeros((128, N_UNITS, 128), np.float32)
    for u, (j, dz, dy, dx) in enumerate(UNITS):
        wk[0:64, u, :] = w[:, :, dz, dy, dx].T
        hz, hy, hx = dz, dy + (j == 1), dx + (j == 0)
        wk[64:128, u, :] = w[:, :, hz, hy, hx].T
    for i, (dz, dy, dx) in enumerate(SINGLES):
        wk[0:64, 12 + i, :] = w[:, :, dz, dy, dx].T
    wk = wk.astype(bf16)

    in_maps = []
    for c in range(N_CORES):
        b, zh = divmod(c, 2)
        z0 = zh * ZS
        xpad = np.zeros((CIN, PW + 1, PW + 1, PW + 1), np.float32)
        xpad[:, 1:25, 1:25, 1:25] = x[b]
        X2 = np.zeros((2, 128, NP, PW, PW), np.float32)
        for j, (sz, sy, sx) in enumerate(((0, 0, 1), (0, 1, 0))):
            X2[j, 0:64] = xpad[:, z0 : z0 + NP, 0:PW, 0:PW]
            X2[j, 64:128] = xpad[
                :, z0 + sz : z0 + sz + NP, sy : sy + PW, sx : sx + PW
            ]
        in_maps.append({"x": X2.astype(bf16), "wk": wk})
    return in_maps


def _gather(results):
    out = np.empty((4, COUT, DHW, DHW, DHW), np.float32)
    for c in range(N_CORES):
        b, zh = divmod(c, 2)
        out[b, :, zh * ZS : (zh + 1) * ZS] = np.asarray(
            results[c]["y"], dtype=np.float32
        )
    return out


def kernel(x, weight):
    from concourse.bass_utils import run_bass_kernel_spmd

    x = np.asarray(x, np.float32)
    in_maps = _make_in_maps(x, weight)
    nc = _build_program()
    res = run_bass_kernel_spmd(nc, in_maps, list(range(N_CORES)))
    return _gather(res.results)


# revision 17
# speedup vs baseline: 1.0086x; 1.0086x over previous
"""Trainium2 Bass kernel: 3D 'same' convolution (implicit GEMM).

Problem: x (4, 64, 24, 24, 24) f32, weight (1, 128, 1728) f32
         -> out (4, 128, 24, 24, 24) f32  (SAME conv3d, k=3)

Sharding (8 cores): batch (4) x z-halves (2). Each core computes
out[b, :, z0:z0+12] for its (b, zh) shard; no inter-core communication.

Per-core algorithm: tap-PAIR-packed implicit GEMM in bf16 (fp32 PSUM).
The PE drains one 128-wide output column per cycle regardless of row
tiling, so throughput is (matmul instructions x N columns) -- the win
is packing TWO conv taps into each 128-deep contraction: partitions
0-63 hold the zero-padded input window, partitions 64-127 hold the
SAME window translated by one voxel, so a single K=128 matmul
accumulates both taps of a pair. Buffer 0 pairs along x (9 pairs),
buffer 1 along y at dx=2 (3 pairs); the 3 leftover taps (dz,2,2) are
K=64 matmuls on the lower half. 15 column-drain units instead of 27.
The shifted replicas are prebuilt host-side so each buffer is a single
whole-tile HBM load.

Loop order is unit-outer over groups of output tiles (one PSUM bank
per tile, 8 banks), so each stationary weight is reused for up to 8
consecutive matmuls; evacuation is a single DVE copy per tile (no
cross-bank add). For timing, the body is unrolled UNROLL x inside the
hardware For_i loop and software-pipelined over NPAR parity buffer
sets: body k prefetches sub-iteration k+2's inputs while computing on
k's, giving loads a two-sub-iteration window.
"""

import sys

if "/opt/trn_rl_repo" not in sys.path:
    sys.path.insert(0, "/opt/trn_rl_repo")

import numpy as np

CIN, COUT, K = 64, 128, 3
DHW = 24  # cubic spatial extent
ZS = 12  # z-planes per shard
NP = 14  # padded z-planes per shard window (ZS + 2 halo)
PW = 26  # padded y/x extent
N_CORES = 8
UNROLL = 8
NPAR = 4  # parity buffer sets (UNROLL % NPAR == 0)

# pair units: (buffer j, lower-tap (dz, dy, dx)); buffer j's upper half
# is shifted by +1 in x (j=0) or y (j=1), pairing the lower tap with
# (dz,dy,dx+1) / (dz,dy+1,dx). Units 12-14 are the unpaired taps
# (dz,2,2), computed as K=64 matmuls on the lower half only.
UNITS = [(0, dz, dy, 0) for dz in range(3) for dy in range(3)] + [
    (1, dz, 0, 2) for dz in range(3)
]
SINGLES = [(0, 2, 2), (1, 2, 2), (2, 2, 2)]
N_UNITS = 15


def _build_program(loop_n=None):
    """Build the SPMD Bass program (one NeuronCore's view).

    loop_n: if set, wrap the whole body in a hardware For_i loop with
    loop_n/UNROLL iterations of UNROLL unrolled bodies (used by test.py
    for wall-clock timing). Must be divisible by UNROLL.
    """
    import concourse.tile as tile
    from concourse import bacc, mybir

    F32 = mybir.dt.float32
    BF16 = mybir.dt.bfloat16

    nc = bacc.Bacc("TRN2")
    x_in = nc.declare_dram_parameter("x", [2, 128, NP, PW, PW], BF16, isOutput=False)
    wk_in = nc.declare_dram_parameter("wk", [128, N_UNITS, 128], BF16, isOutput=False)
    y_out = nc.declare_dram_parameter("y", [128, ZS, DHW, DHW], BF16, isOutput=True)

    with tile.TileContext(nc) as tc:
        with (
            tc.tile_pool(name="wp", bufs=1) as w_pool,
            tc.tile_pool(name="xp", bufs=1) as x_pool,
            tc.tile_pool(name="ps", bufs=8, space="PSUM") as ps_pool,
            tc.tile_pool(name="ob", bufs=4) as ob_pool,
        ):
            # weights are loop-invariant: load once, outside the loop
            W = w_pool.tile([128, N_UNITS, 128], BF16, name="W")
            nc.sync.dma_start(out=W[:], in_=wk_in[:])

            # NPAR explicit buffer sets; the loop body prefetches parity
            # (k+2) % NPAR while computing on parity k % NPAR
            XJP = [
                [
                    x_pool.tile([128, NP, PW, PW], BF16, name=f"XJ{j}p{p}")
                    for j in range(2)
                ]
                for p in range(NPAR)
            ]

            def load_set(XJ):
                # both input loads on the sync ring: the scalar ring's
                # head blocks on evac-gated output DMAs (head-of-line),
                # which would delay a load queued behind them
                nc.sync.dma_start(out=XJ[0][:], in_=x_in[0])
                nc.sync.dma_start(out=XJ[1][:], in_=x_in[1])

            def compute(XJ):
                # output tiles: ("plane", z) N=504 (21x24, 2D AP)
                #           or ("rem", zoff) N=432 (6x3x24, 3D AP)
                groups = (
                    [("plane", z) for z in range(8)],
                    [("plane", z) for z in range(8, 12)] + [("rem", 0), ("rem", 6)],
                )

                def rhs(kind, z, j, dz, dy, dx, lo, hi):
                    if kind == "plane":
                        return XJ[j][lo:hi, z + dz, dy : dy + 21, dx : dx + 24]
                    return XJ[j][
                        lo:hi, z + dz : z + dz + 6, 21 + dy : 24 + dy, dx : dx + 24
                    ]

                for group in groups:
                    n_full = [504 if kind == "plane" else 432 for kind, _ in group]
                    ps = [
                        ps_pool.tile([128, 512], F32, name="ps", tag="ps")
                        for _ in group
                    ]
                    for u in range(N_UNITS):
                        if u < 12:
                            j, dz, dy, dx = UNITS[u]
                            for t, (kind, z) in enumerate(group):
                                nc.tensor.matmul(
                                    ps[t][:, : n_full[t]],
                                    lhsT=W[0:128, u, :],
                                    rhs=rhs(kind, z, j, dz, dy, dx, 0, 128),
                                    start=(u == 0),
                                    stop=False,
                                    skip_group_check=True,
                                )
                        else:
                            # unpaired tap: K=64, lower half only
                            dz, dy, dx = SINGLES[u - 12]
                            for t, (kind, z) in enumerate(group):
                                nc.tensor.matmul(
                                    ps[t][:, : n_full[t]],
                                    lhsT=W[0:64, u, :],
                                    rhs=rhs(kind, z, 0, dz, dy, dx, 0, 64),
                                    start=False,
                                    stop=(u == N_UNITS - 1),
                                    skip_group_check=True,
                                    tile_position=(0, 0),
                                )

                    for t, (kind, z) in enumerate(group):
                        n = n_full[t]
                        # bf16 evacuation: halves out-DMA bytes and doubles
                        # DVE copy throughput; host upcasts to f32
                        ob = ob_pool.tile([128, 512], BF16, name="ob", tag="ob")
                        nc.vector.tensor_copy(ob[:, :n], ps[t][:, :n])
                        if kind == "plane":
                            nc.scalar.dma_start(
                                out=y_out[:, z, 0:21, :], in_=ob[:, :n]
                            )
                        else:
                            nc.scalar.dma_start(
                                out=y_out[:, z : z + 6, 21:24, :], in_=ob[:, :n]
                            )

            if loop_n is not None:
                # pick the largest unroll dividing loop_n; prefetch
                # distance shrinks gracefully for small unrolls
                unroll = next(u for u in (8, 4, 2, 1) if loop_n % u == 0)
                npar = min(NPAR, unroll)
                pf = 2 if npar >= 4 else (1 if npar == 2 else 0)
                for p in range(max(pf, 1)):
                    load_set(XJP[p])
                with tc.For_i(0, loop_n // unroll, 1, staggered_reset=True) as _i:
                    for k in range(unroll):
                        if pf:
                            load_set(XJP[(k + pf) % npar])
                        compute(XJP[k % npar])
                        if not pf:
                            load_set(XJP[0])
            else:
                load_set(XJP[0])
                compute(XJP[0])

    nc.finalize()
    return nc


def _make_in_maps(x, weight):
    import ml_dtypes

    bf16 = ml_dtypes.bfloat16
    w = np.asarray(weight, np.float32).reshape(COUT, CIN, K, K, K)
    wk = np.zeros((128, N_UNITS, 128), np.float32)
    for u, (j, dz, dy, dx) in enumerate(UNITS):
        wk[0:64, u, :] = w[:, :, dz, dy, dx].T
        hz, hy, hx = dz, dy + (j == 1), dx + (j == 0)
        wk[64:128, u, :] = w[:, :, hz, hy, hx].T
    for i, (dz, dy, dx) in enumerate(SINGLES):
        wk[0:64, 12 + i, :] = w[:, :, dz, dy, dx].T
    wk = wk.astype(bf16)

    in_maps = []
    for c in range(N_CORES):
        b, zh = divmod(c, 2)
        z0 = zh * ZS
        xpad = np.zeros((CIN, PW + 1, PW + 1, PW + 1), np.float32)
        xpad[:, 1:25, 1:25, 1:25] = x[b]
        X2 = np.zeros((2, 128, NP, PW, PW), np.float32)
        for j, (sz, sy, sx) in enumerate(((0, 0, 1), (0, 1, 0))):
            X2[j, 0:64] = xpad[:, z0 : z0 + NP, 0:PW, 0:PW]
            X2[j, 64:128] = xpad[
                :, z0 + sz : z0 + sz + NP, sy : sy + PW, sx : sx + PW
            ]
        in_maps.append({"x": X2.astype(bf16), "wk": wk})
    return in_maps


def _gather(results):
    out = np.empty((4, COUT, DHW, DHW, DHW), np.float32)
    for c in range(N_CORES):
        b, zh = divmod(c, 2)
        out[b, :, zh * ZS : (zh + 1) * ZS] = np.asarray(
            results[c]["y"], dtype=np.float32
        )
    return out


def kernel(x, weight):
    from concourse.bass_utils import run_bass_kernel_spmd

    x = np.asarray(x, np.float32)
    in_maps = _make_in_maps(x, weight)
    nc = _build_program()
    res = run_bass_kernel_spmd(nc, in_maps, list(range(N_CORES)))
    return _gather(res.results)
